# revision 8
# baseline (speedup 1.0000x reference)
"""SAGAN-style attention block on 8 Trainium2 NeuronCores.

Math (per batch b):
  theta = W_theta @ x + b_theta            [8, 4096]
  phi   = maxpool2(W_phi @ x + b_phi)      [8, 1024]
  g     = maxpool2(W_g   @ x + b_g)        [32, 1024]
  E[m,n] = exp(S^T[m,n]), S^T[m,n] = sum_c phi[c,m] theta[c,n]
  O_aug = [g; ones] @ E                    [33, 4096]  (row 32 = softmax denom)
  o     = O_aug[0:32] / O_aug[32]
  out   = x + gamma*(W_o @ o + b_o)

Sharding: batch dim (16) split across 8 cores, 2 batches/core; weights
replicated.  No max-subtraction in softmax: |S| <= ~3 so exp is safe, and
the result is mathematically identical.  Matmul operands are bf16 (1 cyc/row
on the PE); accumulation is fp32.

v2 changes vs baseline:
  - S^T matmuls run 2x-concurrent via PE row tiling (32x128 mode, row
    groups 0 and 1): even m-tiles use a pooled-phi replica at partition
    base 0 + the theta copy at proj rows 0:8; odd m-tiles use the
    original operands at base 32.  W_all already carries theta twice.
  - The residual add is folded into the W_o matmul: lhsT is extended
    with a 64x64 identity block and the moving operand carries x (bf16)
    at rows 40:104.  This removes the fp32 x input (halves input DMA)
    and turns the DVE add into a PSUM->SBUF copy.
"""

import ml_dtypes
import numpy as np

import concourse.bass as bass
import concourse.mybir as mybir
import concourse.tile as tile
from concourse import bacc
from concourse.bass_utils import run_bass_kernel_spmd
from concourse.masks import make_identity

B, C, H, W = 16, 64, 64, 64
N = H * W            # 4096 pixels
M = N // 4           # 1024 pooled pixels
NCORES = 8
BPC = B // NCORES    # 2 batches per core
CT = C // 8          # 8 theta/phi channels
CG = C // 2          # 32 g channels
NC = 512             # n-chunk width
NCH = N // NC        # 8 chunks
MT = 128             # m-tile (partitions)
MTS = M // MT        # 8 m-tiles
GRP = 2              # m-tiles per exp group ([128, 1024] PSUM staging)

F32 = mybir.dt.float32
BF16 = mybir.dt.bfloat16
EXP = mybir.ActivationFunctionType.Exp
MAX = mybir.AluOpType.max

KO = 104             # W_o-matmul contraction: 33 o_aug + 7 pad + 64 x


def build_bass(loop_n=None, variant="full"):
    """loop_n: if set, wrap the whole computation in a hardware loop that
    repeats it loop_n times (benchmarking only).
    variant: "full" | "noout" (skip normalize/output tail) |
    "nopost" (skip everything after exp)."""
    import contextlib

    repeat = 1
    if variant.startswith("x"):
        repeat, variant = int(variant[1:]), "full"

    nc = bacc.Bacc("TRN2", target_bir_lowering=False, debug=False)

    # projection output layout (zero-padded for legal partition windows):
    # rows 0:8 + 32:40 theta (row-tiling moving operands), 64:96 g,
    # 96:104 phi.  The pool window 64:104 is one DVE op from base 64;
    # pooled rows land at g 0:32 (base 0, transpose-ready) and phi 32:40
    # (base 32, the row-group-1 stationary).
    xbf_d = nc.dram_tensor("xbf", [BPC, C, N], BF16, kind="ExternalInput").ap()
    wallt_d = nc.dram_tensor("w_all_t", [C, 104], BF16,
                             kind="ExternalInput").ap()
    ball_d = nc.dram_tensor("bias_all", [104, 1], F32,
                            kind="ExternalInput").ap()
    wot_d = nc.dram_tensor("wot_ext", [KO, C], BF16, kind="ExternalInput").ap()
    out_d = nc.dram_tensor("out", [BPC, C, N], F32, kind="ExternalOutput").ap()

    with tile.TileContext(nc) as tc:
        with (
            tc.tile_pool(name="consts", bufs=1) as consts,
            tc.tile_pool(name="perbatch", bufs=2) as pb,
            tc.tile_pool(name="epool", bufs=4) as ep,
            tc.tile_pool(name="small", bufs=3) as sm,
            tc.tile_pool(name="outp", bufs=3) as op_pool,
            tc.tile_pool(name="spsum", bufs=2, space="PSUM") as s_psum,
            tc.tile_pool(name="projpsum", bufs=1, space="PSUM") as sp_proj,
            tc.tile_pool(name="opsum", bufs=2, space="PSUM") as sp,
            tc.tile_pool(name="upsum", bufs=1, space="PSUM") as sp_u,
        ):
            wallt = consts.tile([C, 104], BF16)
            nc.sync.dma_start(out=wallt, in_=wallt_d)
            ball = consts.tile([104, 1], F32)
            nc.sync.dma_start(out=ball, in_=ball_d)
            wot = consts.tile([KO, C], BF16)
            nc.sync.dma_start(out=wot, in_=wot_d)
            ident = consts.tile([CG, CG], BF16)
            make_identity(nc, ident)

            loop_cm = (tc.For_i(0, loop_n, 1) if loop_n
                       else contextlib.nullcontext())
            with loop_cm:
                batch_body(nc, tc, locals(), variant, repeat)
    nc.compile()
    return nc


def batch_body(nc, tc, env, variant="full", repeat=1):
    xbf_d, out_d = env["xbf_d"], env["out_d"]
    wallt, ball, wot, ident = (env["wallt"], env["ball"], env["wot"],
                               env["ident"])
    pb, ep, sm, op_pool, s_psum, sp = (env["pb"], env["ep"], env["sm"],
                                       env["op_pool"], env["s_psum"],
                                       env["sp"])
    sp_proj, sp_u = env["sp_proj"], env["sp_u"]
    for b in list(range(BPC)) * repeat:
        xbf = pb.tile([C, N], BF16, tag="xbf")     # proj matmul rhs
        proj = pb.tile([104, N], BF16, tag="proj")
        # pooled g (rows 0:32) / phi (rows 32:40, row-group-1 stationary)
        pgp = pb.tile([40, M], BF16, tag="pgp")
        phi0 = pb.tile([CT, M], BF16, tag="phi0")  # phi replica, base 0
        gaT = pb.tile([MT, MTS, 33], BF16, tag="gaT")  # g_aug^T tiles
        # W_o-matmul moving operand: rows 0:32 o/denom, 32 ones (b_o),
        # 33:40 zero pad, 40:104 x in bf16 (residual via identity block)
        onorm = pb.tile([KO, N], BF16, tag="onorm")
        outb = pb.tile([C, N], F32, tag="outb")    # output staging

        nc.gpsimd.memset(gaT[:, :, 32], 1.0)    # ones col of g_aug^T
        nc.gpsimd.memset(onorm[32:40, :], 0.0)  # pad rows (0 * garbage)
        nc.gpsimd.memset(onorm[32:33, :], 1.0)  # ones row for b_o

        # batched input loads (keep dma_start count low: SP sequencing is
        # ~0.5-1us per descriptor)
        nc.sync.dma_start(out=xbf[:, 0:N // 2], in_=xbf_d[b][:, 0:N // 2])
        nc.sync.dma_start(out=xbf[:, N // 2:N], in_=xbf_d[b][:, N // 2:N])
        nc.sync.dma_start(out=onorm[40:KO, 0:N // 2],
                          in_=xbf_d[b][:, 0:N // 2])
        nc.sync.dma_start(out=onorm[40:KO, N // 2:N],
                          in_=xbf_d[b][:, N // 2:N])

        # ---- projection phase: theta/phi/g = W_all @ x + bias ----
        for j in range(NCH):
            js = slice(j * NC, (j + 1) * NC)
            pj = sp_proj.tile([104, NC], F32, tag="pj")
            nc.tensor.matmul(pj, wallt, xbf[:, js], start=True, stop=True)
            # PSUM -> SBUF with per-channel bias
            nc.vector.tensor_scalar_add(out=proj[:, js], in0=pj, scalar1=ball)
            # 2x2 maxpool of g/phi rows (chunk j = 8 h-rows x 64 w) in
            # one fused op pair over proj rows 64:104.
            mjs = slice(j * 128, (j + 1) * 128)
            ch = proj[64:104, js].rearrange("p (w t) -> p w t", t=2)
            wm = sm.tile([40, 256], BF16, tag="wm")
            nc.vector.tensor_tensor(out=wm, in0=ch[:, :, 0],
                                    in1=ch[:, :, 1], op=MAX)
            wmv = wm.rearrange("p (h t w) -> p h t w", t=2, w=W // 2)
            po = pgp[:, mjs].rearrange("p (h w) -> p h w", w=W // 2)
            nc.vector.tensor_tensor(out=po, in0=wmv[:, :, 0, :],
                                    in1=wmv[:, :, 1, :], op=MAX)

        # phi replica at partition base 0 for row-group-0 S^T matmuls
        nc.vector.tensor_copy(out=phi0, in_=pgp[32:40, :])

        # ---- transpose pooled g via PE: gaT[:, i, 0:32] = g_pool^T ----
        gt = sp_proj.tile([MT, MTS * CG], BF16, tag="pj")
        for i in range(MTS):
            nc.tensor.transpose(gt[:, i * CG:(i + 1) * CG],
                                pgp[0:CG, i * MT:(i + 1) * MT], ident)
        nc.vector.tensor_copy(
            out=gaT[:, :, 0:32],
            in_=gt.rearrange("p (i c) -> p i c", c=CG))

        # ---- attention phase.  S^T matmuls for the even/odd m-tile of a
        # pair run CONCURRENTLY on PE row groups 0/1 (32x128 tiling): the
        # row-group-0 MM streams theta from proj rows 0:8 against the
        # base-0 phi replica; row-group-1 uses the original base-32
        # operands.  O matmuls of stage s are emitted after the S^T+exp
        # of stage s+1 (software pipelining) so the in-order PE stream
        # never has a head-of-line O waiting on the running exp. ----
        def emit_o(stage):
            et, g, ot = stage
            for t in range(GRP):
                i = GRP * g + t
                nc.tensor.matmul(
                    ot, gaT[:, i, :], et[:, t * NC:(t + 1) * NC],
                    start=(i == 0), stop=(i == MTS - 1))

        def emit_tail(tjs):
            # W_o matmul + PSUM evacuation for an already-normalized chunk.
            # Deferred into the NEXT chunk's group loop so the in-order PE
            # stream never stalls waiting on the DVE normalize chain.
            ut = sp_u.tile([C, NC], F32, tag="ut")
            nc.tensor.matmul(ut, wot, onorm[:, tjs], start=True, stop=True)
            nc.vector.tensor_copy(out=outb[:, tjs], in_=ut)

        tail_mm = None
        for j in range(NCH):
            js = slice(j * NC, (j + 1) * NC)
            ot = sp.tile([33, NC], F32, tag="ot")  # O_aug accumulator
            pending = None
            for g in range(MTS // GRP):
                i0, i1 = GRP * g, GRP * g + 1
                st = s_psum.tile([MT, GRP * NC], F32, tag="st")
                nc.tensor.matmul(
                    st[:, 0:NC],
                    phi0[:, i0 * MT:(i0 + 1) * MT],
                    proj[0:8, js],
                    start=True, stop=True, tile_position=(0, 0))
                nc.tensor.matmul(
                    st[:, NC:2 * NC],
                    pgp[32:40, i1 * MT:(i1 + 1) * MT],
                    proj[32:40, js],
                    start=True, stop=True, tile_position=(32, 0))
                et = ep.tile([MT, GRP * NC], BF16, tag="et")
                nc.scalar.activation(out=et, in_=st, func=EXP)
                if pending is not None:
                    emit_o(pending)
                if g == 2 and tail_mm is not None:
                    emit_tail(tail_mm)
                    tail_mm = None
                pending = (et, g, ot)
            emit_o(pending)
            if variant == "nopost":
                ob0 = op_pool.tile([33, NC], F32, tag="ob")
                nc.vector.tensor_copy(out=ob0, in_=ot)
                nc.sync.dma_start(out=out_d[b][0:33, js], in_=ob0)
                continue
            # normalize: o_norm = O[0:32] * (1/denom), bcast over rows
            # (DVE/GpSimd only -- runs behind the next chunk's PE work)
            rs = sm.tile([1, NC], F32, tag="rs")
            nc.vector.reciprocal(out=rs, in_=ot[32:33, :])
            if variant == "noout":
                nc.sync.dma_start(out=out_d[b][0:1, js], in_=rs)
                continue
            r32 = sm.tile([CG, NC], F32, tag="r32")
            nc.gpsimd.partition_broadcast(r32, rs)
            nc.vector.tensor_tensor(out=onorm[0:32, js], in0=ot[0:32, :],
                                    in1=r32, op=mybir.AluOpType.mult)
            # out = x + gamma*(W_o @ o + b_o): gamma/b_o folded in wot,
            # residual via the identity block against onorm rows 40:104
            tail_mm = js
        if tail_mm is not None:
            emit_tail(tail_mm)
        if variant == "full":
            nc.sync.dma_start(out=out_d[b][:, 0:N // 2],
                              in_=outb[:, 0:N // 2])
            nc.sync.dma_start(out=out_d[b][:, N // 2:N],
                              in_=outb[:, N // 2:N])


_NC_CACHE = None


def _get_nc():
    global _NC_CACHE
    if _NC_CACHE is None:
        _NC_CACHE = build_bass()
    return _NC_CACHE


def make_in_maps(inputs, W_theta, b_theta, W_phi, b_phi, W_g, b_g, W_o, b_o,
                 gamma, **_unused):
    inputs = np.asarray(inputs, np.float32)
    W_all = np.zeros((104, C), np.float32)
    W_all[0:CT] = np.asarray(W_theta, np.float32)
    W_all[32:32 + CT] = np.asarray(W_theta, np.float32)
    W_all[64:64 + CG] = np.asarray(W_g, np.float32)
    W_all[96:96 + CT] = np.asarray(W_phi, np.float32)
    W_all_t = np.ascontiguousarray(W_all.T.astype(ml_dtypes.bfloat16))
    bias_all = np.zeros((104, 1), np.float32)
    bias_all[0:CT, 0] = np.asarray(b_theta, np.float32)
    bias_all[32:32 + CT, 0] = np.asarray(b_theta, np.float32)
    bias_all[64:64 + CG, 0] = np.asarray(b_g, np.float32)
    bias_all[96:96 + CT, 0] = np.asarray(b_phi, np.float32)
    g = np.float32(np.asarray(gamma, np.float32))
    wot_ext = np.zeros((KO, C), np.float32)
    wot_ext[0:CG] = np.asarray(W_o, np.float32).T * g
    wot_ext[CG] = np.asarray(b_o, np.float32) * g
    wot_ext[40:KO] = np.eye(C, dtype=np.float32)
    wot_ext = np.ascontiguousarray(wot_ext.astype(ml_dtypes.bfloat16))

    x = inputs.reshape(B, C, N)
    xbf = x.astype(ml_dtypes.bfloat16)
    in_maps = []
    for c in range(NCORES):
        in_maps.append({
            "xbf": np.ascontiguousarray(xbf[c * BPC:(c + 1) * BPC]),
            "w_all_t": W_all_t,
            "bias_all": bias_all,
            "wot_ext": wot_ext,
        })
    return in_maps


def kernel(**inputs):
    in_maps = make_in_maps(**inputs)
    nc = _get_nc()
    res = run_bass_kernel_spmd(nc, in_maps, core_ids=list(range(NCORES)))
    out = np.concatenate([res.results[c]["out"] for c in range(NCORES)], axis=0)
    return out.reshape(B, C, H, W)


if __name__ == "__main__":
    rng = np.random.default_rng(0)
    ins = {
        "inputs": rng.standard_normal((B, C, H, W)).astype(np.float32),
        "W_theta": (rng.standard_normal((CT, C)) * 0.05).astype(np.float32),
        "b_theta": np.zeros(CT, np.float32),
        "W_phi": (rng.standard_normal((CT, C)) * 0.05).astype(np.float32),
        "b_phi": np.zeros(CT, np.float32),
        "W_g": (rng.standard_normal((CG, C)) * 0.05).astype(np.float32),
        "b_g": np.zeros(CG, np.float32),
        "W_o": (rng.standard_normal((C, CG)) * 0.05).astype(np.float32),
        "b_o": np.zeros(C, np.float32),
        "gamma": np.float32(0.5),
    }
    print(kernel(**ins).shape)
